# revision 7
# baseline (speedup 1.0000x reference)
"""Multi-head attention with bias on 8 TRN2 NeuronCores.

Sharding: zero-collective batch+head sharding. 8 cores = 4 batches x 2
head-groups. Core c handles batch b = c//2 and heads [8*(c%2), 8*(c%2)+8)
for all 1024 query tokens. Each core computes q/k/v projections for its
512 head-dims, biased softmax attention for its 8 heads, and a partial
output projection (contracting only its 512 din rows of Wo). The host
sums the two partial outputs per batch; no cross-core communication.

All matmul inputs are bf16 (host-cast); accumulation is f32 in PSUM.

Per-core device program highlights:
  - scores^T[k, q] matmuls are pair-packed: heads (2p, 2p+1) occupy
    partitions 0-63 / 64-127 of qT/kT block p, so their K=64 matmuls run
    concurrently on disjoint PE row-groups (2x effective throughput).
  - softmax bias folded multiplicatively: host precomputes exp(bias);
    device computes ex = exp(s) * exp(bias) elementwise (DVE, with 2 of
    8 tiles per unit offloaded to GPSIMD).
  - AV uses a replicated-denominator trick: lhsT for head A is
    [v_A | ones] and for head B [ones | v_B], so one M=128 matmul yields
    oT on one 64-partition half and the softmax denominator replicated
    across the other half -- no transposes, and head B's oT lands at
    partitions 64-127, exactly where the output projection needs it.
  - 1/den: DVE partition-offset copy of the replicated denominator to
    the oT-aligned half, then a single reciprocal_approx_fast, then one
    fused multiply per head that reads oT from PSUM and writes bf16 oTn.
  - exp runs as [128, 2048] f32 reads spanning 4 PSUM banks (one ACT
    instruction per 2 k-chunks of a head pair); q/k PSUM->SBUF copies
    and the first output-projection half run on ACT (Copy activation)
    to keep DVE under the PE roofline.
"""

import numpy as np
import ml_dtypes

import concourse.bass as bass
import concourse.mybir as mybir
import concourse.tile as tile
from concourse import bacc
from concourse.bass import ts
from concourse.bass_utils import run_bass_kernel_spmd

F32 = mybir.dt.float32
BF16 = mybir.dt.bfloat16
AF = mybir.ActivationFunctionType
BF = ml_dtypes.bfloat16

B, N, D = 4, 1024, 1024
H, HD = 16, 64
HL = 8            # heads per core
NC = 8            # 1024 / 128 chunks
P = 128
NQT = 512         # qtile width

_CACHE = {}


def _build():
    nc = bacc.Bacc("TRN2", target_bir_lowering=False, debug=False,
                   enable_asserts=False, num_devices=8)
    xT_d = nc.dram_tensor("xT", [P, NC, N], BF16, kind="ExternalInput").ap()
    wq_d = nc.dram_tensor("wq", [P, 4, NC, P], BF16, kind="ExternalInput").ap()
    wk_d = nc.dram_tensor("wk", [P, 4, NC, P], BF16, kind="ExternalInput").ap()
    wv_d = nc.dram_tensor("wv", [P, NC, 512], BF16, kind="ExternalInput").ap()
    wo_d = nc.dram_tensor("wo", [P, 4, D], BF16, kind="ExternalInput").ap()
    eb_d = nc.dram_tensor("eb", [HL, P, NC, N], BF16, kind="ExternalInput").ap()
    out_d = nc.dram_tensor("out", [N, D], BF16, kind="ExternalOutput").ap()

    with tile.TileContext(nc) as tc:
        with tc.tile_pool(name="big", bufs=1) as big_pool, \
             tc.tile_pool(name="w", bufs=4) as w_pool, \
             tc.tile_pool(name="eb", bufs=4) as eb_pool, \
             tc.tile_pool(name="es", bufs=2) as es_pool, \
             tc.tile_pool(name="ex", bufs=4) as ex_pool, \
             tc.tile_pool(name="rec", bufs=2) as rec_pool, \
             tc.tile_pool(name="sc", bufs=1, space="PSUM") as sc_pool, \
             tc.tile_pool(name="av", bufs=2, space="PSUM") as av_pool, \
             tc.tile_pool(name="pp", bufs=2, space="PSUM") as pp_pool:

            xt = big_pool.tile([P, NC, N], BF16)
            wq_t = w_pool.tile([P, 4, NC, P], BF16, tag="w")
            wk_t = w_pool.tile([P, 4, NC, P], BF16, tag="w")
            wv_t = w_pool.tile([P, NC, 512], BF16, tag="w")
            wo_t = w_pool.tile([P, 4, D], BF16, tag="w")
            # sync queue: what the first projections need, in need-order;
            # bulk (v/o weights) rides the idle gpsimd SWDGE queue.
            nc.sync.dma_start(wq_t[:], wq_d[:])
            for cc in range(NC):
                nc.sync.dma_start(xt[:, cc, :], xT_d[:, cc, :])
            nc.sync.dma_start(wk_t[:], wk_d[:])
            nc.gpsimd.dma_start(wv_t[:], wv_d[:])
            nc.gpsimd.dma_start(wo_t[:], wo_d[:])

            qT = big_pool.tile([P, 4, N], BF16)   # [dout%128, block, tok]
            kT = big_pool.tile([P, 4, N], BF16)
            # v_aug per (chunk, head): even h -> [v | ones], odd h -> [ones | v]
            v_sb = big_pool.tile([P, NC, HL, P], BF16)
            nc.gpsimd.memset(v_sb[:], 1.0)
            oTn = big_pool.tile([P, 4, N], BF16)  # normalized heads^T
            out_sb = big_pool.tile([P, NC, D], BF16)

            eb_t = [None] * HL

            def fetch_eb(h):
                eb_t[h] = eb_pool.tile([P, NC, N], BF16, tag="eb",
                                       name=f"eb{h}")
                nc.gpsimd.dma_start(eb_t[h][:], eb_d[h, :, :, :])

            fetch_eb(0)
            fetch_eb(1)

            def q_proj(m, t):
                ps = pp_pool.tile([P, NQT], F32, tag="pp", name=f"q{m}_{t}")
                for cc in range(NC):
                    nc.tensor.matmul(ps[:], wq_t[:, m, cc, :],
                                     xt[:, cc, ts(t, NQT)],
                                     start=(cc == 0), stop=(cc == NC - 1))
                nc.scalar.copy(qT[:, m, ts(t, NQT)], ps[:])

            def k_proj(m, t):
                ps = pp_pool.tile([P, NQT], F32, tag="pp", name=f"k{m}_{t}")
                for cc in range(NC):
                    nc.tensor.matmul(ps[:], wk_t[:, m, cc, :],
                                     xt[:, cc, ts(t, NQT)],
                                     start=(cc == 0), stop=(cc == NC - 1))
                nc.scalar.copy(kT[:, m, ts(t, NQT)], ps[:])

            def v_proj(kc):
                ps = pp_pool.tile([P, 512], F32, tag="pp", name=f"v{kc}")
                for cc in range(NC):
                    nc.tensor.matmul(ps[:], xt[:, cc, ts(kc, P)],
                                     wv_t[:, cc, :],
                                     start=(cc == 0), stop=(cc == NC - 1))
                # even heads -> cols 0-63 of their slot; odd -> cols 64-127
                nc.vector.tensor_copy(
                    v_sb[:, kc, 0:HL:2, 0:HD],
                    ps[:].rearrange("p (h d) -> p h d", h=HL)[:, 0:HL:2, :])
                nc.vector.tensor_copy(
                    v_sb[:, kc, 1:HL:2, HD:P],
                    ps[:].rearrange("p (h d) -> p h d", h=HL)[:, 1:HL:2, :])

            def out_proj(tb, dt, glo, ghi, first):
                ps = pp_pool.tile([P, NQT], F32, tag="pp",
                                  name=f"op{tb}_{dt}_{glo}")
                for g in range(glo, ghi):
                    nc.tensor.matmul(ps[:], oTn[:, g, ts(tb, P)],
                                     wo_t[:, g, ts(dt, NQT)],
                                     start=(g == glo), stop=(g == ghi - 1))
                if first:
                    nc.scalar.copy(out_sb[:, tb, ts(dt, NQT)], ps[:])
                else:
                    nc.vector.tensor_add(out_sb[:, tb, ts(dt, NQT)], ps[:],
                                         out_sb[:, tb, ts(dt, NQT)])

            # ---- PE filler queue ----
            filler = []
            fi = 0

            def fill(n):
                nonlocal fi
                done = 0
                while fi < len(filler) and done < n:
                    filler[fi]()
                    fi += 1
                    done += 1

            # upfront: q/k for unit (0, 0) only
            q_proj(0, 0)
            k_proj(0, 0)

            filler.append(lambda: q_proj(0, 1))
            filler.append(lambda: k_proj(0, 1))
            for kc in range(NC):
                filler.append(lambda kc=kc: v_proj(kc))
            for m in range(1, 4):
                for t in range(2):
                    filler.append(lambda m=m, t=t: q_proj(m, t))
                    filler.append(lambda m=m, t=t: k_proj(m, t))

            def unit(p, t):
                """One (pair, qtile): scores -> exp -> bias-mult -> AV ->
                reciprocal -> fused normalize."""
                o_acc_a = av_pool.tile([P, NQT], F32, tag="av",
                                       name=f"oa{p}_{t}")
                o_acc_b = av_pool.tile([P, NQT], F32, tag="av",
                                       name=f"ob{p}_{t}")
                ha, hb = 2 * p, 2 * p + 1
                pend = []

                def flush():
                    exa, exb, g = pend.pop(0)
                    for j in range(2):
                        k = 2 * g + j
                        nc.tensor.matmul(o_acc_a[:], v_sb[:, k, ha, :],
                                         exa[:, j, :],
                                         start=(k == 0), stop=(k == NC - 1))
                        nc.tensor.matmul(o_acc_b[:], v_sb[:, k, hb, :],
                                         exb[:, j, :],
                                         start=(k == 0), stop=(k == NC - 1))

                for g in range(4):
                    sc = sc_pool.tile([P, 4, NQT], F32, tag="sc",
                                      name=f"sc{p}_{t}_{g}")
                    for j in range(2):
                        k = 2 * g + j
                        nc.tensor.matmul(sc[:, j, :],
                                         kT[0:HD, p, ts(k, P)],
                                         qT[0:HD, p, ts(t, NQT)],
                                         start=True, stop=True)
                        nc.tensor.matmul(sc[:, 2 + j, :],
                                         kT[HD:P, p, ts(k, P)],
                                         qT[HD:P, p, ts(t, NQT)],
                                         start=True, stop=True)
                    es = es_pool.tile([P, 4, NQT], BF16, tag="es")
                    nc.scalar.activation(es[:], sc[:], AF.Exp)
                    exa = ex_pool.tile([P, 2, NQT], BF16, tag="ex")
                    exb = ex_pool.tile([P, 2, NQT], BF16, tag="ex")
                    nc.vector.tensor_mul(
                        exa[:], es[:, 0:2, :],
                        eb_t[ha][:, 2 * g:2 * g + 2, ts(t, NQT)])
                    eng = nc.gpsimd if g % 2 else nc.vector
                    eng.tensor_mul(
                        exb[:], es[:, 2:4, :],
                        eb_t[hb][:, 2 * g:2 * g + 2, ts(t, NQT)])
                    pend.append((exa, exb, g))
                    fill(2)
                    if len(pend) > 1:
                        flush()
                while pend:
                    fill(2)
                    flush()
                # denominators (replicated on the complementary halves) ->
                # oT-aligned copies -> one reciprocal -> fused normalize
                stg = rec_pool.tile([P, NQT], F32, tag="stg",
                                    name=f"stg{p}_{t}")
                rec = rec_pool.tile([P, NQT], F32, tag="rec",
                                    name=f"rec{p}_{t}")
                nc.vector.tensor_copy(stg[0:HD, :], o_acc_a[HD:P, :])
                nc.vector.tensor_copy(stg[HD:P, :], o_acc_b[0:HD, :])
                nc.vector.reciprocal_approx_fast(rec[:], stg[:])
                nc.vector.tensor_mul(oTn[0:HD, p, ts(t, NQT)],
                                     o_acc_a[0:HD, :], rec[0:HD, :])
                nc.vector.tensor_mul(oTn[HD:P, p, ts(t, NQT)],
                                     o_acc_b[HD:P, :], rec[HD:P, :])

            for p in range(4):
                if p < 3:
                    fetch_eb(2 * p + 2)
                    fetch_eb(2 * p + 3)
                for t in range(2):
                    unit(p, t)
                if p == 1:
                    # pairs 0-1 done: their out-proj half becomes filler
                    for tb in range(NC):
                        for dt in range(2):
                            filler.append(
                                lambda tb=tb, dt=dt: out_proj(tb, dt, 0, 2,
                                                              True))
            fill(len(filler))

            # tail: contract pairs 2-3 and store
            for tb in range(NC):
                for dt in range(2):
                    out_proj(tb, dt, 2, 4, False)
                nc.sync.dma_start(out_d[ts(tb, P), :], out_sb[:, tb, :])

    nc.compile()
    return nc


def _prep_in_maps(x, attn_bias, Wq, Wk, Wv, Wo):
    x = np.asarray(x, dtype=np.float32)
    attn_bias = np.asarray(attn_bias, dtype=np.float32)
    scale = float(HD) ** -0.5

    def _wqk(w, sc):
        # [dout, din] -> wq[p, m, cc, j] = (W.T*sc)[cc*128+p, hg*512+m*128+j]
        wt = (np.asarray(w, dtype=np.float32).T * sc)      # [din, dout]
        a = wt.reshape(NC, P, 2, 4, P)                     # [cc, p, hg, m, j]
        return np.ascontiguousarray(a.transpose(2, 1, 3, 0, 4)).astype(BF)

    def _wv(w):
        wt = np.asarray(w, dtype=np.float32).T             # [din, dout]
        a = wt.reshape(NC, P, 2, 512)                      # [cc, p, hg, j]
        return np.ascontiguousarray(a.transpose(2, 1, 0, 3)).astype(BF)

    def _wo(w):
        wt = np.asarray(w, dtype=np.float32).T             # [din, dout]
        a = wt.reshape(2, 4, P, D)                         # [hg, g, p, j]
        return np.ascontiguousarray(a.transpose(0, 2, 1, 3)).astype(BF)

    wq_a = _wqk(Wq, scale)
    wk_a = _wqk(Wk, 1.0)
    wv_a = _wv(Wv)
    wo_a = _wo(Wo)

    xT = {}
    for b in range(B):
        a = x[b].T.reshape(NC, P, N)                       # [cc, p, tok]
        xT[b] = np.ascontiguousarray(a.transpose(1, 0, 2)).astype(BF)

    in_maps = []
    for core in range(8):
        b, hg = core // 2, core % 2
        ebs = np.exp(attn_bias[b, 8 * hg:8 * hg + 8])      # [8, q, k]
        a = ebs.transpose(0, 2, 1).reshape(HL, NC, P, N)   # [h, kc, p, q]
        eb = np.ascontiguousarray(a.transpose(0, 2, 1, 3)).astype(BF)
        in_maps.append({"xT": xT[b], "wq": wq_a[hg], "wk": wk_a[hg],
                        "wv": wv_a[hg], "wo": wo_a[hg], "eb": eb})
    return in_maps


def _unshard(res):
    out = np.empty((B, N, D), dtype=np.float32)
    for b in range(B):
        out[b] = (np.asarray(res.results[2 * b]["out"], dtype=np.float32)
                  + np.asarray(res.results[2 * b + 1]["out"],
                               dtype=np.float32))
    return out


def kernel(x, attn_bias, Wq, Wk, Wv, Wo):
    if "nc" not in _CACHE:
        _CACHE["nc"] = _build()
    in_maps = _prep_in_maps(x, attn_bias, Wq, Wk, Wv, Wo)
    _CACHE["in_maps"] = in_maps
    res = run_bass_kernel_spmd(_CACHE["nc"], in_maps, core_ids=list(range(8)))
    return _unshard(res)


def run_traced(inputs):
    """Profiled run (test harness only; needs the antenv ntff hook shim)."""
    if "nc" not in _CACHE:
        _CACHE["nc"] = _build()
    in_maps = _CACHE.get("in_maps") or _prep_in_maps(**inputs)
    return run_bass_kernel_spmd(_CACHE["nc"], in_maps,
                                core_ids=list(range(8)), trace=True)


# revision 8
# speedup vs baseline: 1.2539x; 1.2539x over previous
"""Multi-head attention with bias on 8 TRN2 NeuronCores.

Sharding: zero-collective batch+head sharding. 8 cores = 4 batches x 2
head-groups. Core c handles batch b = c//2 and heads [8*(c%2), 8*(c%2)+8)
for all 1024 query tokens. Each core computes q/k/v projections for its
512 head-dims, biased softmax attention for its 8 heads, and a partial
output projection (contracting only its 512 din rows of Wo). The host
sums the two partial outputs per batch; no cross-core communication.

All matmul inputs are bf16 (host-cast); accumulation is f32 in PSUM.

Per-core device program highlights:
  - scores^T[k, q] matmuls are pair-packed: heads (2p, 2p+1) occupy
    partitions 0-63 / 64-127 of qT/kT block p, so their K=64 matmuls run
    concurrently on disjoint PE row-groups (2x effective throughput).
  - softmax bias folded multiplicatively: host precomputes exp(bias);
    device computes ex = exp(s) * exp(bias) elementwise (DVE, with 2 of
    8 tiles per unit offloaded to GPSIMD).
  - AV uses a replicated-denominator trick: lhsT for head A is
    [v_A | ones] and for head B [ones | v_B], so one M=128 matmul yields
    oT on one 64-partition half and the softmax denominator replicated
    across the other half -- no transposes, and head B's oT lands at
    partitions 64-127, exactly where the output projection needs it.
  - 1/den: DVE partition-offset copy of the replicated denominator to
    the oT-aligned half, then a single reciprocal_approx_fast, then one
    fused multiply per head that reads oT from PSUM and writes bf16 oTn.
  - exp runs as [128, 2048] f32 reads spanning 4 PSUM banks (one ACT
    instruction per 2 k-chunks of a head pair); q/k PSUM->SBUF copies
    and the first output-projection half run on ACT (Copy activation)
    to keep DVE under the PE roofline.
"""

import numpy as np
import ml_dtypes

import concourse.bass as bass
import concourse.mybir as mybir
import concourse.tile as tile
from concourse import bacc
from concourse.bass import ts
from concourse.bass_utils import run_bass_kernel_spmd

F32 = mybir.dt.float32
BF16 = mybir.dt.bfloat16
AF = mybir.ActivationFunctionType
BF = ml_dtypes.bfloat16

B, N, D = 4, 1024, 1024
H, HD = 16, 64
HL = 8            # heads per core
NC = 8            # 1024 / 128 chunks
P = 128
NQT = 512         # qtile width

_CACHE = {}


def _build():
    nc = bacc.Bacc("TRN2", target_bir_lowering=False, debug=False,
                   enable_asserts=False, num_devices=8)
    xT_d = nc.dram_tensor("xT", [P, NC, N], BF16, kind="ExternalInput").ap()
    wq_d = nc.dram_tensor("wq", [P, 4, NC, P], BF16, kind="ExternalInput").ap()
    wk_d = nc.dram_tensor("wk", [P, 4, NC, P], BF16, kind="ExternalInput").ap()
    wv_d = nc.dram_tensor("wv", [P, NC, 512], BF16, kind="ExternalInput").ap()
    wo_d = nc.dram_tensor("wo", [P, 4, D], BF16, kind="ExternalInput").ap()
    eb_d = nc.dram_tensor("eb", [HL, P, NC, N], BF16, kind="ExternalInput").ap()
    out_d = nc.dram_tensor("out", [N, D], BF16, kind="ExternalOutput").ap()

    with tile.TileContext(nc) as tc:
        with tc.tile_pool(name="big", bufs=1) as big_pool, \
             tc.tile_pool(name="w", bufs=4) as w_pool, \
             tc.tile_pool(name="eb", bufs=4) as eb_pool, \
             tc.tile_pool(name="es", bufs=2) as es_pool, \
             tc.tile_pool(name="ex", bufs=4) as ex_pool, \
             tc.tile_pool(name="rec", bufs=2) as rec_pool, \
             tc.tile_pool(name="sc", bufs=1, space="PSUM") as sc_pool, \
             tc.tile_pool(name="av", bufs=2, space="PSUM") as av_pool, \
             tc.tile_pool(name="pp", bufs=2, space="PSUM") as pp_pool:

            xt = big_pool.tile([P, NC, N], BF16)
            wq_t = w_pool.tile([P, 4, NC, P], BF16, tag="w")
            wk_t = w_pool.tile([P, 4, NC, P], BF16, tag="w")
            wv_t = w_pool.tile([P, NC, 512], BF16, tag="w")
            wo_t = w_pool.tile([P, 4, D], BF16, tag="w")
            # sync queue: what the first projections need, in need-order;
            # bulk (v/o weights) rides the idle gpsimd SWDGE queue.
            nc.sync.dma_start(wq_t[:], wq_d[:])
            for cc in range(NC):
                nc.sync.dma_start(xt[:, cc, :], xT_d[:, cc, :])
            nc.sync.dma_start(wk_t[:], wk_d[:])
            nc.gpsimd.dma_start(wv_t[:], wv_d[:])
            nc.gpsimd.dma_start(wo_t[:], wo_d[:])

            qT = big_pool.tile([P, 4, N], BF16)   # [dout%128, block, tok]
            kT = big_pool.tile([P, 4, N], BF16)
            # v_aug per (chunk, head): even h -> [v | ones], odd h -> [ones | v]
            v_sb = big_pool.tile([P, NC, HL, P], BF16)
            nc.gpsimd.memset(v_sb[:], 1.0)
            oTn = big_pool.tile([P, 4, N], BF16)  # normalized heads^T
            out_sb = big_pool.tile([P, NC, D], BF16)

            eb_t = [None] * HL

            def fetch_eb(h):
                eb_t[h] = eb_pool.tile([P, NC, N], BF16, tag="eb",
                                       name=f"eb{h}")
                nc.gpsimd.dma_start(eb_t[h][:], eb_d[h, :, :, :])

            fetch_eb(0)
            fetch_eb(1)

            def q_proj(m, t):
                ps = pp_pool.tile([P, NQT], F32, tag="pp", name=f"q{m}_{t}")
                for cc in range(NC):
                    nc.tensor.matmul(ps[:], wq_t[:, m, cc, :],
                                     xt[:, cc, ts(t, NQT)],
                                     start=(cc == 0), stop=(cc == NC - 1))
                nc.scalar.copy(qT[:, m, ts(t, NQT)], ps[:])

            def k_proj(m, t):
                ps = pp_pool.tile([P, NQT], F32, tag="pp", name=f"k{m}_{t}")
                for cc in range(NC):
                    nc.tensor.matmul(ps[:], wk_t[:, m, cc, :],
                                     xt[:, cc, ts(t, NQT)],
                                     start=(cc == 0), stop=(cc == NC - 1))
                nc.scalar.copy(kT[:, m, ts(t, NQT)], ps[:])

            def v_proj(kc):
                ps = pp_pool.tile([P, 512], F32, tag="pp", name=f"v{kc}")
                for cc in range(NC):
                    nc.tensor.matmul(ps[:], xt[:, cc, ts(kc, P)],
                                     wv_t[:, cc, :],
                                     start=(cc == 0), stop=(cc == NC - 1))
                # even heads -> cols 0-63 of their slot; odd -> cols 64-127
                nc.vector.tensor_copy(
                    v_sb[:, kc, 0:HL:2, 0:HD],
                    ps[:].rearrange("p (h d) -> p h d", h=HL)[:, 0:HL:2, :])
                nc.vector.tensor_copy(
                    v_sb[:, kc, 1:HL:2, HD:P],
                    ps[:].rearrange("p (h d) -> p h d", h=HL)[:, 1:HL:2, :])

            def out_proj(tb, dt, glo, ghi, first):
                ps = pp_pool.tile([P, NQT], F32, tag="pp",
                                  name=f"op{tb}_{dt}_{glo}")
                for g in range(glo, ghi):
                    nc.tensor.matmul(ps[:], oTn[:, g, ts(tb, P)],
                                     wo_t[:, g, ts(dt, NQT)],
                                     start=(g == glo), stop=(g == ghi - 1))
                if first:
                    nc.scalar.copy(out_sb[:, tb, ts(dt, NQT)], ps[:])
                else:
                    nc.vector.tensor_add(out_sb[:, tb, ts(dt, NQT)], ps[:],
                                         out_sb[:, tb, ts(dt, NQT)])

            # ---- PE filler queue ----
            filler = []
            fi = 0

            def fill(n):
                nonlocal fi
                done = 0
                while fi < len(filler) and done < n:
                    filler[fi]()
                    fi += 1
                    done += 1

            # upfront: q/k for unit (0, 0) only
            q_proj(0, 0)
            k_proj(0, 0)

            filler.append(lambda: q_proj(0, 1))
            filler.append(lambda: k_proj(0, 1))
            for kc in range(NC):
                filler.append(lambda kc=kc: v_proj(kc))
            for m in range(1, 4):
                for t in range(2):
                    filler.append(lambda m=m, t=t: q_proj(m, t))
                    filler.append(lambda m=m, t=t: k_proj(m, t))

            def unit(p, t):
                """One (pair, qtile): scores -> exp -> bias-mult -> AV ->
                reciprocal -> fused normalize."""
                o_acc_a = av_pool.tile([P, NQT], F32, tag="av",
                                       name=f"oa{p}_{t}")
                o_acc_b = av_pool.tile([P, NQT], F32, tag="av",
                                       name=f"ob{p}_{t}")
                ha, hb = 2 * p, 2 * p + 1
                pend = []

                def flush():
                    exa, exb, g = pend.pop(0)
                    for j in range(2):
                        k = 2 * g + j
                        nc.tensor.matmul(o_acc_a[:], v_sb[:, k, ha, :],
                                         exa[:, j, :],
                                         start=(k == 0), stop=(k == NC - 1))
                        nc.tensor.matmul(o_acc_b[:], v_sb[:, k, hb, :],
                                         exb[:, j, :],
                                         start=(k == 0), stop=(k == NC - 1))

                for g in range(4):
                    sc = sc_pool.tile([P, 4, NQT], F32, tag="sc",
                                      name=f"sc{p}_{t}_{g}")
                    for j in range(2):
                        k = 2 * g + j
                        nc.tensor.matmul(sc[:, j, :],
                                         kT[0:HD, p, ts(k, P)],
                                         qT[0:HD, p, ts(t, NQT)],
                                         start=True, stop=True)
                        nc.tensor.matmul(sc[:, 2 + j, :],
                                         kT[HD:P, p, ts(k, P)],
                                         qT[HD:P, p, ts(t, NQT)],
                                         start=True, stop=True)
                    es = es_pool.tile([P, 4, NQT], BF16, tag="es")
                    nc.scalar.activation(es[:], sc[:], AF.Exp)
                    exa = ex_pool.tile([P, 2, NQT], BF16, tag="ex")
                    exb = ex_pool.tile([P, 2, NQT], BF16, tag="ex")
                    nc.vector.tensor_mul(
                        exa[:], es[:, 0:2, :],
                        eb_t[ha][:, 2 * g:2 * g + 2, ts(t, NQT)])
                    nc.vector.tensor_mul(
                        exb[:], es[:, 2:4, :],
                        eb_t[hb][:, 2 * g:2 * g + 2, ts(t, NQT)])
                    pend.append((exa, exb, g))
                    fill(2)
                    if len(pend) > 1:
                        flush()
                while pend:
                    fill(2)
                    flush()
                # denominators (replicated on the complementary halves) ->
                # oT-aligned copies -> one reciprocal -> fused normalize
                stg = rec_pool.tile([P, NQT], F32, tag="stg",
                                    name=f"stg{p}_{t}")
                rec = rec_pool.tile([P, NQT], F32, tag="rec",
                                    name=f"rec{p}_{t}")
                nc.vector.tensor_copy(stg[0:HD, :], o_acc_a[HD:P, :])
                nc.vector.tensor_copy(stg[HD:P, :], o_acc_b[0:HD, :])
                nc.vector.reciprocal_approx_fast(rec[:], stg[:])
                nc.vector.tensor_mul(oTn[0:HD, p, ts(t, NQT)],
                                     o_acc_a[0:HD, :], rec[0:HD, :])
                nc.vector.tensor_mul(oTn[HD:P, p, ts(t, NQT)],
                                     o_acc_b[HD:P, :], rec[HD:P, :])

            for p in range(4):
                if p < 3:
                    fetch_eb(2 * p + 2)
                    fetch_eb(2 * p + 3)
                for t in range(2):
                    unit(p, t)
                if p == 1:
                    # pairs 0-1 done: their out-proj half becomes filler
                    for tb in range(NC):
                        for dt in range(2):
                            filler.append(
                                lambda tb=tb, dt=dt: out_proj(tb, dt, 0, 2,
                                                              True))
            fill(len(filler))

            # tail: contract pairs 2-3 and store
            for tb in range(NC):
                for dt in range(2):
                    out_proj(tb, dt, 2, 4, False)
                nc.sync.dma_start(out_d[ts(tb, P), :], out_sb[:, tb, :])

    nc.compile()
    return nc


def _prep_in_maps(x, attn_bias, Wq, Wk, Wv, Wo):
    x = np.asarray(x, dtype=np.float32)
    attn_bias = np.asarray(attn_bias, dtype=np.float32)
    scale = float(HD) ** -0.5

    def _wqk(w, sc):
        # [dout, din] -> wq[p, m, cc, j] = (W.T*sc)[cc*128+p, hg*512+m*128+j]
        wt = (np.asarray(w, dtype=np.float32).T * sc)      # [din, dout]
        a = wt.reshape(NC, P, 2, 4, P)                     # [cc, p, hg, m, j]
        return np.ascontiguousarray(a.transpose(2, 1, 3, 0, 4)).astype(BF)

    def _wv(w):
        wt = np.asarray(w, dtype=np.float32).T             # [din, dout]
        a = wt.reshape(NC, P, 2, 512)                      # [cc, p, hg, j]
        return np.ascontiguousarray(a.transpose(2, 1, 0, 3)).astype(BF)

    def _wo(w):
        wt = np.asarray(w, dtype=np.float32).T             # [din, dout]
        a = wt.reshape(2, 4, P, D)                         # [hg, g, p, j]
        return np.ascontiguousarray(a.transpose(0, 2, 1, 3)).astype(BF)

    wq_a = _wqk(Wq, scale)
    wk_a = _wqk(Wk, 1.0)
    wv_a = _wv(Wv)
    wo_a = _wo(Wo)

    xT = {}
    for b in range(B):
        a = x[b].T.reshape(NC, P, N)                       # [cc, p, tok]
        xT[b] = np.ascontiguousarray(a.transpose(1, 0, 2)).astype(BF)

    in_maps = []
    for core in range(8):
        b, hg = core // 2, core % 2
        ebs = np.exp(attn_bias[b, 8 * hg:8 * hg + 8])      # [8, q, k]
        a = ebs.transpose(0, 2, 1).reshape(HL, NC, P, N)   # [h, kc, p, q]
        eb = np.ascontiguousarray(a.transpose(0, 2, 1, 3)).astype(BF)
        in_maps.append({"xT": xT[b], "wq": wq_a[hg], "wk": wk_a[hg],
                        "wv": wv_a[hg], "wo": wo_a[hg], "eb": eb})
    return in_maps


def _unshard(res):
    out = np.empty((B, N, D), dtype=np.float32)
    for b in range(B):
        out[b] = (np.asarray(res.results[2 * b]["out"], dtype=np.float32)
                  + np.asarray(res.results[2 * b + 1]["out"],
                               dtype=np.float32))
    return out


def kernel(x, attn_bias, Wq, Wk, Wv, Wo):
    if "nc" not in _CACHE:
        _CACHE["nc"] = _build()
    in_maps = _prep_in_maps(x, attn_bias, Wq, Wk, Wv, Wo)
    _CACHE["in_maps"] = in_maps
    res = run_bass_kernel_spmd(_CACHE["nc"], in_maps, core_ids=list(range(8)))
    return _unshard(res)


def run_traced(inputs):
    """Profiled run (test harness only; needs the antenv ntff hook shim)."""
    if "nc" not in _CACHE:
        _CACHE["nc"] = _build()
    in_maps = _CACHE.get("in_maps") or _prep_in_maps(**inputs)
    return run_bass_kernel_spmd(_CACHE["nc"], in_maps,
                                core_ids=list(range(8)), trace=True)
